# revision 51
# baseline (speedup 1.0000x reference)
"""Trainium2 Bass kernel for BiAttention (b=8, n=m=1024, d=512).

Sharding: data-parallel over batch — one batch element per NeuronCore,
8 cores, no cross-core communication.

v3 design (evolves v2; same math, restructured schedule):

  x1T  (d,n) = transpose(cast_bf16(x1))          [PE transpose, bf16 —
  x2Tw (d,m) = transpose(cast_bf16(x2)) * w3      2x faster than v2's f32
               ++ col m=w1                        transposes]
  sim row t  = x1T_t^T @ x2Tw -> psum [512 | 257], s1 col gains logm1 bias
  E row t    = exp(psum) bf16; col vm = g1
  ET         = DMA-XBAR transpose of each E row as it completes
  u_row(t)   = ET_t^T @ x2g -> (n,d|den1); c2q=U/den1  -> out cols [0:2D]
  u_col(u)   = E_u^T @ x1g  -> (m,d|den2); Q2C=U*(g2/den2)
  v_row(t)   = ET_t^T @ Q2C; q2c_att=V/den1            -> out cols [2D:3D]

Differences vs v2:
  * out block 0 (== x1 verbatim) is assembled on the HOST during unshard;
    the device only writes [c2q | x1*c2q | x1*q2c_att] (3D wide) — saves
    2MB/core of pure-copy DMA writes.
  * per-row pipeline: each sim row immediately exps, XBAR-transposes and
    feeds u_row(t), so the 6MB of output DMA spreads over the whole
    kernel instead of piling into the last 20us.
  * input DMAs issue from the Sync queue (idle early) instead of Act;
    E transposes issue from the Pool queue so they don't FIFO behind
    output writes on Sync's queue.
  * transposes run at bf16 rate (casts on DVE), evictions rebalanced:
    x1T->Pool, x2Tw/E/combo->Act, Q2C->Pool.

Mask-suffix specialization as v2: NEFF compiled per (kn, km) kept-tile
counts; partially-masked tiles exact via exponent biases.
"""

import numpy as np
from contextlib import ExitStack

import concourse.bacc as bacc
import concourse.tile as tile
import concourse.mybir as mybir
from concourse.bass_utils import run_bass_kernel_spmd
from concourse.masks import make_identity

F32 = mybir.dt.float32
BF = mybir.dt.bfloat16
U8 = mybir.dt.uint8
EXP = mybir.ActivationFunctionType.Exp
COPY = mybir.ActivationFunctionType.Copy

P = 128
N = 1024          # x1 rows
M = 1024          # x2 rows
D = 512           # feature dim
NT, MT, DC = N // P, M // P, D // P
NEGB = -30000.0   # exp(x + NEGB) == 0.0 exactly for |x| < 80

N_CORES = 8

_CACHE = {}


def _chunks(width, lim=512):
    out = []
    o = 0
    while o < width:
        w = min(lim, width - o)
        out.append((o, w))
        o += w
    return out


def _build(kn, km):
    """Build the kernel keeping the first kn n-tiles / km m-tiles of the
    contractions (tiles beyond that must be fully masked)."""
    vm = km * P  # valid m extent
    nc = bacc.Bacc("TRN2", target_bir_lowering=False, debug=False)
    x1d = nc.dram_tensor("x1", [N, D], F32, kind="ExternalInput").ap()
    x2d = nc.dram_tensor("x2", [M, D], F32, kind="ExternalInput").ap()
    m1d = nc.dram_tensor("x1_mask", [N], U8, kind="ExternalInput").ap()
    m2d = nc.dram_tensor("x2_mask", [M], U8, kind="ExternalInput").ap()
    wd = nc.dram_tensor("W", [3 * D], F32, kind="ExternalInput").ap()
    outd = nc.dram_tensor("out", [N, 3 * D], F32, kind="ExternalOutput").ap()

    x1r_d = x1d.rearrange("(t p) d -> p t d", p=P)
    x2r_d = x2d.rearrange("(t p) d -> p t d", p=P)
    out_r = outd.rearrange("(t p) e -> p t e", p=P)

    # sim psum chunks over the widened (vm+1) extent; last chunk carries s1
    mch = _chunks(vm + 1)
    mpair = (km + 1) // 2

    with tile.TileContext(nc) as tc, ExitStack() as ctx:
        const = ctx.enter_context(tc.tile_pool(name="const", bufs=1))
        big = ctx.enter_context(tc.tile_pool(name="big", bufs=1))
        rows = ctx.enter_context(tc.tile_pool(name="rows", bufs=1))
        work = ctx.enter_context(tc.tile_pool(name="work", bufs=4))
        psA = ctx.enter_context(tc.tile_pool(name="psA", bufs=2, space="PSUM"))
        psB = ctx.enter_context(tc.tile_pool(name="psB", bufs=2, space="PSUM"))
        psC = ctx.enter_context(tc.tile_pool(name="psC", bufs=2, space="PSUM"))
        pstb = ctx.enter_context(tc.tile_pool(name="pstb", bufs=2, space="PSUM"))

        # ---------- big buffers ----------
        x1n = big.tile([P, NT, D], F32)        # natural x1 (products)
        x2n = big.tile([P, km, D], F32)        # natural x2
        x1b = big.tile([P, NT, D], BF)         # bf16 casts (transpose srcs)
        x2b = big.tile([P, km, D], BF)
        x1g = big.tile([P, kn, D + 1], BF)     # x1*g1 ++ g1 col
        x2g = big.tile([P, km, D + 1], BF)     # x2*g2 ++ g2 col
        x1T = big.tile([P, DC, N], BF)         # (d_chunk, n)
        x2Tw = big.tile([P, DC, vm + 1], BF)   # (d_chunk, m)*w3 ++ w1 col
        E = big.tile([P, NT, vm + 1], BF)      # exp(sim); col vm = g1
        ET = big.tile([P, km, N], BF)          # E^T
        Q2C = big.tile([P, km, D], BF)         # q2c * g2

        # ---------- identities FIRST on the Pool queue ----------
        identb = const.tile([P, P], BF)
        make_identity(nc, identb)
        identf = const.tile([P, P], F32)
        make_identity(nc, identf)

        # ---------- input DMAs: big tensors ALL on the Sync HWDGE queue
        # in strict need-order — one queue drains FIFO at full aggregate
        # bandwidth; splitting pairs across two queues halves each
        # stream's rate. Tiny rows (w, masks) ride Act's queue. ----------
        wrow = rows.tile([1, 12 * P], F32)
        nc.sync.dma_start(wrow[:], wd.rearrange("(a n) -> a n", a=1))
        m1row = rows.tile([1, N], U8)
        nc.sync.dma_start(m1row[:], m1d.rearrange("(a n) -> a n", a=1))
        m2row = rows.tile([1, M], U8)
        nc.sync.dma_start(m2row[:], m2d.rearrange("(a n) -> a n", a=1))
        nc.sync.dma_start(x1n[:, 0:2, :], x1r_d[:, 0:2, :])
        nc.sync.dma_start(x2n[:, 0:2, :], x2r_d[:, 0:2, :])
        nc.sync.dma_start(x2n[:, 2:min(4, km), :], x2r_d[:, 2:min(4, km), :])
        nc.sync.dma_start(x1n[:, 2:4, :], x1r_d[:, 2:4, :])
        for p in range(2, mpair):
            hi = min(2 * p + 2, km)
            nc.sync.dma_start(x2n[:, 2 * p:hi, :], x2r_d[:, 2 * p:hi, :])
        for p in range(2, (NT + 1) // 2):
            nc.sync.dma_start(x1n[:, 2 * p:2 * p + 2, :],
                              x1r_d[:, 2 * p:2 * p + 2, :])

        onef = const.tile([1, 1], F32)
        nc.vector.memset(onef[:], 1.0)

        # W row -> columns via PE row->col transposes
        pwc = psC.tile([P, 12], F32, tag="psC")
        for c in range(12):
            nc.tensor.transpose(pwc[:, c:c + 1], wrow[0:1, c * P:(c + 1) * P],
                                onef[0:1, 0:1])
        wcols = const.tile([P, 12], F32)  # (p, c): w1=0:4 w2=4:8 w3=8:12
        nc.vector.tensor_copy(wcols[:], pwc[:])
        w3rec = const.tile([P, 4], F32)
        nc.vector.reciprocal(w3rec[:], wcols[:, 8:12])
        u2r = const.tile([P, 4], BF)      # w2/w3 — recovers s2 from x2Tw
        nc.vector.tensor_mul(u2r[:], wcols[:, 4:8], w3rec[:])

        # ---------- bf16 casts (DVE), per DMA pair ----------
        def x1_cast(p):
            nc.vector.tensor_copy(x1b[:, 2 * p:2 * p + 2, :],
                                  x1n[:, 2 * p:2 * p + 2, :])

        def x2_cast(p):
            hi = min(2 * p + 2, km)
            nc.vector.tensor_copy(x2b[:, 2 * p:hi, :], x2n[:, 2 * p:hi, :])

        # ---------- PE transposes. Pair 0 of each tensor transposes in
        # f32 straight from the naturals (PE is idle that early, and it
        # skips the cast latency); later pairs go through the bf16 casts
        # at 2x the PE rate. ----------
        def _tq_pool(c):
            # alternate quad psum pools by chunk parity: with a single
            # 2-buf pool the third quad's transposes wait on the FIRST
            # quad's (serialized, slower) eviction — PE head-of-line
            return (psC, "psC") if c % 2 == 0 else (pstb, "pst")

        def x1_pair_T(p):
            f32 = p == 0
            src, ident = (x1n, identf) if f32 else (x1b, identb)
            dt = F32 if f32 else BF
            for c in range(DC):
                pool, tag = _tq_pool(c)
                pq = pool.tile([P, 256], dt, tag=tag, name=f"x1p_{p}_{c}")
                for j in range(2):
                    nc.tensor.transpose(pq[:, j * P:(j + 1) * P],
                                        src[:, 2 * p + j, c * P:(c + 1) * P],
                                        ident[:])
                nc.vector.tensor_copy(x1T[:, c, p * 256:(p + 1) * 256], pq[:])

        def x1_tile_xbar(tile):
            # later x1 tiles transpose through the DMA XBAR on Sync (idle
            # mid-phase-1) — zero PE/DVE cost, same pattern as e_xpose
            nc.sync.dma_start(x1T[:, 0:DC, tile * P:(tile + 1) * P],
                              x1b[:, tile, :], transpose=True)

        def x2_pair_T(p):
            jw = min(2, km - 2 * p)
            f32 = p == 0
            src, ident = (x2n, identf) if f32 else (x2b, identb)
            dt = F32 if f32 else BF
            for c in range(DC):
                pool, tag = _tq_pool(c)
                pq = pool.tile([P, 256], dt, tag=tag, name=f"x2p_{p}_{c}")
                for j in range(jw):
                    nc.tensor.transpose(pq[:, j * P:(j + 1) * P],
                                        src[:, 2 * p + j, c * P:(c + 1) * P],
                                        ident[:])
                # evict fused with w3 scaling (per-partition in (d, m) layout)
                nc.scalar.activation(x2Tw[:, c, p * 256:p * 256 + jw * P],
                                     pq[:, 0:jw * P], COPY,
                                     scale=wcols[:, 8 + c:9 + c])

        def w1_cols():
            for c in range(DC):
                nc.vector.tensor_copy(x2Tw[:, c, vm:vm + 1],
                                      wcols[:, c:c + 1])

        # masks -> exponent offsets (0 valid / NEGB padded); x1's become
        # per-partition columns (PE row->col in the idle head window)
        # feeding the s1 exp-evict bias directly
        def mask_rows():
            logm1r = rows.tile([1, N], F32)
            nc.vector.tensor_scalar_mul(logm1r[:], m1row[:], NEGB)
            pm1 = psC.tile([P, NT], F32, tag="psC", name="pm1")
            for t in range(NT):
                nc.tensor.transpose(pm1[:, t:t + 1],
                                    logm1r[0:1, t * P:(t + 1) * P],
                                    onef[0:1, 0:1])
            nc.vector.tensor_copy(logm1c[:], pm1[:])
            nc.vector.tensor_scalar_mul(logm2[:], m2row[0:1, 0:vm], NEGB)

        logm1c = const.tile([P, NT], F32)
        logm2 = rows.tile([1, vm], F32)

        # ---------- s2/g2 path, then x2g ----------
        g2c = const.tile([P, km], F32)

        def s2_g2():
            brow = rows.tile([1, vm], F32)
            for h, (off, w) in enumerate(_chunks(vm)):
                ps_s = psA.tile([1, w], F32, tag="psA", name=f"ps_b2_{h}")
                for c in range(DC):
                    nc.tensor.matmul(ps_s[:], u2r[:, c:c + 1],
                                     x2Tw[:, c, off:off + w],
                                     start=(c == 0), stop=(c == DC - 1))
                nc.vector.tensor_add(brow[:, off:off + w], ps_s[:],
                                     logm2[:, off:off + w])
            pbc = psC.tile([P, km], F32, tag="psC", name="pbc")
            for k in range(km):
                nc.tensor.transpose(pbc[:, k:k + 1],
                                    brow[0:1, k * P:(k + 1) * P],
                                    onef[0:1, 0:1])
            nc.scalar.activation(g2c[:], pbc[:], EXP)

        def x2_gate(k):
            nc.vector.tensor_scalar_mul(x2g[:, k, 0:D], x2n[:, k, :],
                                        g2c[:, k:k + 1])
            nc.vector.tensor_copy(x2g[:, k, D:D + 1], g2c[:, k:k + 1])

        # ---------- sim chunk -> E (exp evict; the s1 column's x1-mask
        # offset rides the activation bias instead of a PE matmul) ----------
        def sim_chunk(t, h):
            off, w = mch[h]
            last = off + w == vm + 1
            pool = psA if w > 320 else psB
            pe = pool.tile([P, w], F32, tag=pool.name, name=f"pe_{t}_{h}")
            for c in range(DC):
                nc.tensor.matmul(pe[:],
                                 x1T[:, c, t * P:(t + 1) * P],
                                 x2Tw[:, c, off:off + w],
                                 start=(c == 0), stop=(c == DC - 1))
            if last:
                nc.scalar.activation(E[:, t, off:off + w - 1], pe[:, 0:w - 1],
                                     EXP)
                nc.scalar.activation(E[:, t, vm:vm + 1], pe[:, w - 1:w],
                                     EXP, bias=logm1c[:, t:t + 1])
            else:
                nc.scalar.activation(E[:, t, off:off + w], pe[:], EXP)

        def sim_row(t):
            for h in range(len(mch)):
                sim_chunk(t, h)

        g1c = const.tile([P, NT], F32)

        def x1_gate(t):
            # x1g = x1 * g1 ++ g1 col (fused scale + f32->bf16 cast)
            g1 = g1c[:, t:t + 1]
            nc.vector.tensor_copy(g1, E[:, t, vm:vm + 1])
            nc.vector.tensor_scalar_mul(x1g[:, t, 0:D], x1n[:, t, :], g1)
            nc.vector.tensor_copy(x1g[:, t, D:D + 1], g1)

        # ---------- E transposes -> ET. The XBAR transpose runs as a
        # ~1.3us ucode instruction ON the issuing engine, so it must live
        # on Sync (idle) — on Act it head-of-line blocks the evictions ----------
        def e_xpose(t):
            nc.sync.dma_start(ET[:, 0:km, t * P:(t + 1) * P],
                              E[:, t, 0:vm], transpose=True)

        # ---------- U_row -> c2q ; out cols [0:2D] ----------
        rden1c = const.tile([P, NT], F32)

        def u_row(t):
            pa = psC.tile([P, 256], F32, tag="psC", name=f"ra_{t}")
            pb = psB.tile([P, 257], F32, tag="psB", name=f"rb_{t}")
            for k in range(km):
                lhs = ET[:, k, t * P:(t + 1) * P]
                nc.tensor.matmul(pa[:], lhs, x2g[:, k, 0:256],
                                 start=(k == 0), stop=(k == km - 1))
                nc.tensor.matmul(pb[:], lhs, x2g[:, k, 256:513],
                                 start=(k == 0), stop=(k == km - 1))
            rd = rden1c[:, t:t + 1]
            nc.vector.reciprocal(rd, pb[:, 256:257])
            combo = work.tile([P, 2 * D], F32, tag="ev", name=f"cb_{t}")
            nc.scalar.activation(combo[:, 0:256], pa[:], COPY, scale=rd)
            nc.scalar.activation(combo[:, 256:512], pb[:, 0:256], COPY,
                                 scale=rd)
            nc.vector.tensor_mul(combo[:, D:2 * D], x1n[:, t, :],
                                 combo[:, 0:D])
            nc.sync.dma_start(out_r[:, t, 0:2 * D], combo[:])

        # ---------- U_col -> Q2C (scaled by g2/den2). Uses pstb/psA
        # banks (idle after phase 1) so its psum rotation never couples
        # with the u_rows' psC/psB rotation. ----------
        def u_col(u):
            pa = pstb.tile([P, 256], F32, tag="pst", name=f"ua_{u}")
            pb = psA.tile([P, 257], F32, tag="psA", name=f"ub_{u}")
            for k in range(kn):
                lhs = E[:, k, u * P:(u + 1) * P]
                nc.tensor.matmul(pa[:], lhs, x1g[:, k, 0:256],
                                 start=(k == 0), stop=(k == kn - 1))
                nc.tensor.matmul(pb[:], lhs, x1g[:, k, 256:513],
                                 start=(k == 0), stop=(k == kn - 1))
            rg = work.tile([P, 1], F32, tag="rg", name=f"rg_{u}")
            nc.vector.reciprocal(rg[:], pb[:, 256:257])
            nc.vector.tensor_mul(rg[:], rg[:], g2c[:, u:u + 1])
            nc.vector.tensor_scalar_mul(Q2C[:, u, 0:256], pa[:], rg[:])
            nc.vector.tensor_scalar_mul(Q2C[:, u, 256:512], pb[:, 0:256],
                                        rg[:])

        # ---------- V -> q2c_att ; out cols [2D:3D] = x1 . (V*rden1) ----------
        def v_row(t):
            # alternate psum pools for a 4-deep eviction pipeline (the
            # bank-padded bufs fit the 512-wide tile); psB first — psA is
            # still held by the last u_col's pb at V-entry
            pool = psB if t % 2 == 0 else psA
            pv = pool.tile([P, D], F32, tag=pool.name, name=f"pv_{t}")
            if t >= NT - 2:
                # tail tiles: precompute x1*rden1 on DVE DURING the matmuls,
                # then a single psum-read mul — drops the Act hop from the
                # critical tail chain
                x1rd = work.tile([P, D], F32, tag="x1rd", name=f"xr_{t}")
                nc.vector.tensor_scalar_mul(x1rd[:], x1n[:, t, :],
                                            rden1c[:, t:t + 1])
            for k in range(km):
                nc.tensor.matmul(pv[:], ET[:, k, t * P:(t + 1) * P],
                                 Q2C[:, k, :],
                                 start=(k == 0), stop=(k == km - 1))
            prod = work.tile([P, D], F32, tag="pr", name=f"pv2_{t}")
            if t >= NT - 2:
                nc.vector.tensor_mul(prod[:], x1rd[:], pv[:])
            else:
                vtmp = work.tile([P, D], F32, tag="vt", name=f"vt_{t}")
                nc.scalar.activation(vtmp[:], pv[:], COPY,
                                     scale=rden1c[:, t:t + 1])
                nc.gpsimd.tensor_mul(prod[:], vtmp[:], x1n[:, t, :])
            nc.sync.dma_start(out_r[:, t, 2 * D:3 * D], prod[:])

        # ---------------- schedule ----------------
        # Phase 1: casts + transposes chase the input DMAs; h0 sim chunks
        # of rows 0-3 fill the window while x2's tail pair and x1's later
        # pairs are still in flight; per-row exp evict + XBAR e_xpose
        # (XBAR writes ET while PE only reads x1T/x2Tw — no contention).
        mask_rows()
        x1_pair_T(0)
        x2_pair_T(0)
        x2_cast(1)
        x2_pair_T(1)
        x1_cast(1)
        x1_tile_xbar(2)
        x1_tile_xbar(3)
        # h0 needs only x2 pairs 0-1; rows 0-3 need x1 pairs 0-1
        sim_chunk(0, 0)
        sim_chunk(1, 0)
        sim_chunk(2, 0)
        sim_chunk(3, 0)
        for p in range(2, mpair):
            x2_cast(p)
            x2_pair_T(p)
        w1_cols()
        s2_g2()
        x1_cast(2)
        x1_tile_xbar(4)
        x1_tile_xbar(5)
        for k in range(km):
            x2_gate(k)
        sim_chunk(0, 1); x1_gate(0); e_xpose(0)
        sim_chunk(1, 1); x1_gate(1); e_xpose(1)
        sim_chunk(2, 1); x1_gate(2); e_xpose(2)
        x1_cast(3)
        x1_tile_xbar(6)
        x1_tile_xbar(7)
        sim_chunk(3, 1); x1_gate(3); e_xpose(3)
        sim_row(4)
        if kn > 4:
            x1_gate(4)
        e_xpose(4)
        sim_row(5)
        if kn > 5:
            x1_gate(5)
        e_xpose(5)
        sim_row(6); e_xpose(6)
        sim_row(7); e_xpose(7)

        # Phase 2: u_rows (DMA-emitting) interleaved with u_cols (no DMA)
        # so the 512KB combo stores are spaced ~2.7us — wider than their
        # transfer time, so Sync never stalls on semaphore reuse.
        u_row(0)
        u_row(1)
        u_col(0)
        u_row(2)
        u_col(1)
        u_row(3)
        u_col(2)
        u_row(4)
        u_col(3)
        u_row(5)
        u_row(6)
        u_col(4)
        u_row(7)
        for u in range(5, km):
            u_col(u)

        # Phase 3: V rows (prod stores are small, 256KB each)
        for t in range(NT):
            v_row(t)

    nc.compile()
    return nc


def _kept_tiles(mask):
    """Tiles (of 128) up to and including the last one with any valid row."""
    valid = ~mask.astype(bool)           # (b, L)
    any_valid = valid.reshape(valid.shape[0], -1, P).any(axis=2).any(axis=0)
    nz = np.nonzero(any_valid)[0]
    return int(nz[-1]) + 1 if len(nz) else 1


def _get_nc(kn, km):
    key = (kn, km)
    if key not in _CACHE:
        _CACHE[key] = _build(kn, km)
    return _CACHE[key]


def _run(inputs, trace=False, trace_cores=None):
    x1 = np.ascontiguousarray(np.asarray(inputs["x1"], dtype=np.float32))
    x2 = np.ascontiguousarray(np.asarray(inputs["x2"], dtype=np.float32))
    m1 = np.ascontiguousarray(np.asarray(inputs["x1_mask"]).astype(np.uint8))
    m2 = np.ascontiguousarray(np.asarray(inputs["x2_mask"]).astype(np.uint8))
    W = np.ascontiguousarray(np.asarray(inputs["W"], dtype=np.float32))
    nc = _get_nc(_kept_tiles(m1), _kept_tiles(m2))
    in_maps = [
        {"x1": x1[i], "x2": x2[i], "x1_mask": m1[i], "x2_mask": m2[i], "W": W}
        for i in range(N_CORES)
    ]
    res = run_bass_kernel_spmd(nc, in_maps, core_ids=list(range(N_CORES)),
                               trace=trace, trace_cores=trace_cores)
    dev = np.stack([res.results[i]["out"] for i in range(N_CORES)], axis=0)
    # out block 0 is x1 verbatim — assembled here during the unshard/gather
    out = np.concatenate([x1, dev], axis=-1)
    return out.astype(np.float32), res


def kernel(x1, x1_mask, x2, x2_mask, W, bias=None, **_kw):
    # bias is mathematically irrelevant: a global additive constant cancels in
    # both softmaxes, and every output term is softmax-weighted.
    out, _ = _run({"x1": x1, "x1_mask": x1_mask, "x2": x2, "x2_mask": x2_mask,
                   "W": W})
    return out
